# revision 1
# baseline (speedup 1.0000x reference)
"""ExpertLinear (dense MoE blend) Trainium2 kernel.

y[b,o] = sum_k ew[b,k] * (x[b,:] @ W[k,o,:]) + sum_k ew[b,k] * bias[k,o]

Data-parallel over B across 8 cores; each core streams the whole blended
weight tensor. Layout/precision choices:
  - Host pre-transposes W -> wT[k, i, o] (contraction dim i on partitions,
    fully contiguous per-partition DMA rows) and casts it to bf16, with 32
    zero columns appended per row block. bf16 halves the dominant HBM
    stream (32MB -> ~17MB per core) and - critically - lets all 16 weight
    tiles stay live in SBUF at once: no buffer reuse means no DMA needs
    both a WAW and WAR wait, which matters because this walrus build
    accepts at most ONE sync wait per instruction.
  - All small operands (xT i-tiles, ew columns replicated across
    partitions, ewT, bias) are packed host-side into one fp32 tensor `xe`
    and arrive via a single DMA (single semaphore lane).
  - VectorE pre-scales xs_k[i,b] = x[b,i] * ew[b,k] into bf16; the PE then
    accumulates the fp32 bias matmul (ewT.T @ bias, K=8) plus all 128
    bf16 W matmuls into 2 PSUM banks, evicted once at the end.
  - Per W tile, a zero-result matmul (wt-slice.T @ zero-column) absorbs
    the tile's DMA-lane wait on its own PE instruction, so the real
    matmuls carry at most their single DVE-tick wait.
Per-core HBM traffic ~= 18.5 MB; measured numerics ~2e-3 relative L2.
"""

import numpy as np

B, E, IN, OUT = 512, 8, 1024, 1024
NCORES = 8
BL = B // NCORES  # 64 rows per core
P = 128
NIT = IN // P  # 8 i-tiles
W_DMA_ITILES = 16  # i-tiles per W DMA
OUTP = OUT + 32  # zero-padded row length in the bf16 W stream
NTILES = (E * NIT) // W_DMA_ITILES  # 16 weight tiles, all live in SBUF

# xe column layout (float32, 128 partitions)
XT_C = 0                       # 8 i-tiles of xT: [128, 8*64]
EWB_C = XT_C + NIT * BL        # ew columns replicated: [128, 8*64]
EWT_C = EWB_C + E * BL         # ewT on partitions 0..7: [8, 64]
BIAS_C = EWT_C + BL            # bias on partitions 0..7: [8, 1024]
XE_COLS = BIAS_C + OUT

_compiled = None


def _patch_drain_split():
    """The walrus build in this container rejects any instruction carrying
    more than one sync wait, including the kernel-tail Drain that
    TileContext emits with one wait per active semaphore. Split it into a
    sequence of single-wait drains (sequencer-FIFO keeps them ordered;
    the set of waits is identical)."""
    import concourse.tile as tile_mod

    if getattr(tile_mod.TileContext, "_drain_split_patched", False):
        return
    from concourse.tile_sem_assignment import N_PROCS
    from concourse.vector_clock import ScopedClock, VectorClock

    def _drain_and_barrier(self, tick_clock, wait_clock):
        gc = tick_clock.global_clock
        for p in range(N_PROCS):
            t = gc[p]
            if t <= 0:
                continue
            ticks = [0] * N_PROCS
            ticks[p] = t
            di = self.nc.sync.drain()
            wait_clock.add_sem_waits(
                di.ins, ScopedClock({None: VectorClock(ticks)})
            )
        self.nc.all_engine_barrier()
        assert self.sems is not None
        popped = self.nc._tile_sem_poison_stack.pop()
        assert popped is self._sem_poison
        self.nc.clear_and_free_semaphores(list(self.sems.allocated().values()))
        self.nc.all_engine_barrier()

    tile_mod.TileContext._drain_and_barrier = _drain_and_barrier
    tile_mod.TileContext._drain_split_patched = True


def _build():
    import concourse.bass as bass
    import concourse.mybir as mybir
    import concourse.tile as tile

    _patch_drain_split()

    f32 = mybir.dt.float32
    bf16 = mybir.dt.bfloat16

    nc = bass.Bass()
    xe_d = nc.dram_tensor("xe", [P, XE_COLS], f32, kind="ExternalInput")
    wT_d = nc.dram_tensor("wT", [E, IN, OUTP], bf16, kind="ExternalInput")
    y_d = nc.dram_tensor("y", [BL, OUT], f32, kind="ExternalOutput")

    with tile.TileContext(nc) as tc:
        with (
            tc.tile_pool(name="const", bufs=1) as const,
            tc.tile_pool(name="wpool", bufs=1) as wpool,
            tc.tile_pool(name="psum", bufs=2, space="PSUM") as psum,
        ):
            xe = const.tile([P, XE_COLS], f32)
            xs = const.tile([P, E * NIT * BL], bf16)
            y_sb = const.tile([BL, OUT], f32)
            wts = [
                wpool.tile([P, W_DMA_ITILES * OUTP], bf16,
                           name=f"wt{t}", tag=f"wt{t}")
                for t in range(NTILES)
            ]

            nc.sync.dma_start(xe[:], xe_d[:])

            # xs_k[i, b] = xT[i, b] * ew[b, k], downcast to bf16
            for k in range(E):
                for ib in range(NIT):
                    nc.vector.tensor_tensor(
                        xs[:, (k * NIT + ib) * BL:(k * NIT + ib + 1) * BL],
                        xe[:, XT_C + ib * BL:XT_C + (ib + 1) * BL],
                        xe[:, EWB_C + k * BL:EWB_C + (k + 1) * BL],
                        mybir.AluOpType.mult,
                    )

            ps0 = psum.tile([BL, 512], f32)
            ps1 = psum.tile([BL, 512], f32)
            ewt_ap = xe[0:E, EWT_C:EWT_C + BL]
            # bias term: y += ewT.T @ bias (K=8, fp32 matmul - only 2 of them)
            nc.tensor.matmul(
                ps0[:], ewt_ap, xe[0:E, BIAS_C:BIAS_C + 512],
                start=True, stop=False,
            )
            nc.tensor.matmul(
                ps1[:], ewt_ap, xe[0:E, BIAS_C + 512:BIAS_C + 1024],
                start=True, stop=False,
            )

            # wT viewed as a flat stream of E*NIT [128, OUTP] i-blocks,
            # grouped W_DMA_ITILES per DMA/tile.
            wT_flat = wT_d[:].rearrange("k (n p) o -> (k n) p o", p=P)
            for t in range(NTILES):
                wt = wts[t]
                src = wT_flat[t * W_DMA_ITILES:(t + 1) * W_DMA_ITILES].rearrange(
                    "n p o -> p n o"
                )
                dst = wt[:].rearrange("p (n o) -> p n o", n=W_DMA_ITILES)
                nc.sync.dma_start(dst, src)
                # zero matmul: wt-slice.T @ zero-column adds 0 to ps0 but
                # absorbs this tile's DMA-lane wait on its own PE
                # instruction (one-sync-wait walrus limit); its ready-set
                # is a subset of the real matmuls' and its priority is
                # earlier, so it schedules first.
                nc.tensor.matmul(
                    ps0[:, 0:1],
                    wt[:, 0:BL],
                    wt[:, OUT:OUT + 1],
                    start=False, stop=False,
                )
                for j in range(W_DMA_ITILES):
                    blk = t * W_DMA_ITILES + j   # global i-block = k*NIT+ib
                    lhsT = xs[:, blk * BL:(blk + 1) * BL]
                    last = blk == E * NIT - 1
                    nc.tensor.matmul(
                        ps0[:], lhsT,
                        wt[:, j * OUTP:j * OUTP + 512],
                        start=False, stop=last,
                    )
                    nc.tensor.matmul(
                        ps1[:], lhsT,
                        wt[:, j * OUTP + 512:j * OUTP + 1024],
                        start=False, stop=last,
                    )

            nc.vector.tensor_copy(y_sb[:, 0:512], ps0[:])
            nc.vector.tensor_copy(y_sb[:, 512:1024], ps1[:])
            nc.sync.dma_start(y_d[:], y_sb[:])

    return nc


def _get_compiled():
    global _compiled
    if _compiled is None:
        _compiled = _build()
    return _compiled


_wT_cache = None


def _make_in_maps(x, expert_weights, weight, bias):
    global _wT_cache
    import ml_dtypes

    if _wT_cache is None or _wT_cache[0] is not weight:
        wT = np.zeros((E, IN, OUTP), dtype=ml_dtypes.bfloat16)
        wT[:, :, :OUT] = (
            np.asarray(weight, dtype=np.float32)
            .transpose(0, 2, 1)
            .astype(ml_dtypes.bfloat16)
        )
        _wT_cache = (weight, wT)
    wT = _wT_cache[1]
    bias = np.ascontiguousarray(np.asarray(bias, dtype=np.float32))
    x = np.asarray(x, dtype=np.float32)
    ew = np.asarray(expert_weights, dtype=np.float32)
    in_maps = []
    for c in range(NCORES):
        xl = x[c * BL:(c + 1) * BL]          # [64, IN]
        ewl = ew[c * BL:(c + 1) * BL]        # [64, E]
        xe = np.zeros((P, XE_COLS), dtype=np.float32)
        xT = xl.T.reshape(NIT, P, BL)        # [8, 128, 64]
        xe[:, XT_C:XT_C + NIT * BL] = xT.transpose(1, 0, 2).reshape(P, NIT * BL)
        ewb = np.broadcast_to(ewl.T[:, None, :], (E, P, BL))  # [8, 128, 64]
        xe[:, EWB_C:EWB_C + E * BL] = ewb.transpose(1, 0, 2).reshape(P, E * BL)
        xe[0:E, EWT_C:EWT_C + BL] = ewl.T
        xe[0:E, BIAS_C:BIAS_C + OUT] = bias
        in_maps.append({"xe": xe, "wT": wT})
    return in_maps


def kernel(x, expert_weights, weight, bias, _trace=False):
    from concourse.bass_utils import run_bass_kernel_spmd

    nc = _get_compiled()
    in_maps = _make_in_maps(x, expert_weights, weight, bias)
    res = run_bass_kernel_spmd(
        nc, in_maps, core_ids=list(range(NCORES)), trace=_trace
    )
    y = np.concatenate([r["y"] for r in res.results], axis=0).astype(np.float32)
    if _trace:
        return y, res
    return y



# revision 11
# speedup vs baseline: 2.1998x; 2.1998x over previous
"""ExpertLinear (dense MoE blend) Trainium2 kernel — expert-sharded.

y[b,o] = sum_k ew[b,k] * (x[b,:] @ W[k,o,:]) + sum_k ew[b,k] * bias[k,o]

Sharding: one expert per core (E == 8 == NCORES). Each core computes its
expert's full GEMM z_c = x @ W[c].T for ALL B rows, scales by ew[:, c] on
eviction, and writes a bf16 partial; the host sums the 8 partials and adds
the (tiny) bias term. This reads each expert's weights exactly once
chip-wide: per-core HBM traffic is ~4 MB (vs ~18.5 MB for data-parallel),
turning a DMA-bound kernel into a PE-bound one (~13.7 us of bf16 matmul).

Layout/precision:
  - Host packs, per core, an interleaved stream of 8 i-chunks; chunk n =
    [wT tile n | xT tile n] as one [128, 1536] bf16 DMA, so the PE's
    i-major loop starts after one chunk lands and each matmul group needs
    exactly ONE sync wait (this walrus build rejects >1 wait/instruction).
  - ew column arrives fp32 [128, 4]; scaling happens on eviction via
    per-partition tensor_scalar_mul (DVE) / activation Copy scale (ACT),
    split across both engines so the eviction tail halves.
  - PSUM: all 8 banks hold the [512, 1024] fp32 partial (4 b-chunks x 2
    o-halves); i-major accumulation, single eviction per bank.
  - A few zero-matmuls at the start keep the PE busy during the DMA
    lead-in so the HAM clock-gate un-throttles (1.2 -> 2.4 GHz) sooner.
"""

import numpy as np

B, E, IN, OUT = 512, 8, 1024, 1024
NCORES = 8
P = 128
NIT = IN // P      # 8 i-tiles (contraction chunks)
BT = B // P        # 4 b-chunks (output partition tiles)
NH = OUT // 512    # 2 o-halves (PSUM bank free-dim limit)
CW = OUT + B  # 1536 cols per i-tile: wT tile (1024) + xT tile (512)
XOFF = OUT          # x region offset inside an i-tile block
N_DUMMY = 4
# i-tile ranges per DMA chunk: small first chunks let the PE start early,
# and 5 input chunks + ew + 2 outputs = 8 DMAs = one per DMAHW lane
CHUNKS = [(0, 1), (1, 2), (2, 4), (4, 6), (6, 8)]

_compiled = None


def _patch_drain_split():
    """The walrus build in this container rejects any instruction carrying
    more than one sync wait, including the kernel-tail Drain that
    TileContext emits with one wait per active semaphore. Split it into a
    sequence of single-wait drains (sequencer-FIFO keeps them ordered;
    the set of waits is identical)."""
    import concourse.tile as tile_mod

    if getattr(tile_mod.TileContext, "_drain_split_patched", False):
        return
    from concourse.tile_sem_assignment import N_PROCS
    from concourse.vector_clock import ScopedClock, VectorClock

    def _drain_and_barrier(self, tick_clock, wait_clock):
        gc = tick_clock.global_clock
        for p in range(N_PROCS):
            t = gc[p]
            if t <= 0:
                continue
            ticks = [0] * N_PROCS
            ticks[p] = t
            di = self.nc.sync.drain()
            wait_clock.add_sem_waits(
                di.ins, ScopedClock({None: VectorClock(ticks)})
            )
        self.nc.all_engine_barrier()
        assert self.sems is not None
        popped = self.nc._tile_sem_poison_stack.pop()
        assert popped is self._sem_poison
        self.nc.clear_and_free_semaphores(list(self.sems.allocated().values()))
        self.nc.all_engine_barrier()

    tile_mod.TileContext._drain_and_barrier = _drain_and_barrier
    tile_mod.TileContext._drain_split_patched = True


def _build():
    import concourse.bass as bass
    import concourse.mybir as mybir
    import concourse.tile as tile

    _patch_drain_split()

    f32 = mybir.dt.float32
    bf16 = mybir.dt.bfloat16
    Copy = mybir.ActivationFunctionType.Copy

    nc = bass.Bass()
    wx_d = nc.dram_tensor("wx", [NIT * P, CW], bf16, kind="ExternalInput")
    ew_d = nc.dram_tensor("ew", [P, BT], f32, kind="ExternalInput")
    yv_d = nc.dram_tensor("yv", [P, BT * 512], bf16, kind="ExternalOutput")
    ya_d = nc.dram_tensor("ya", [P, BT * 512], bf16, kind="ExternalOutput")

    with tile.TileContext(nc) as tc:
        with (
            tc.tile_pool(name="sb", bufs=1) as sb,
            tc.tile_pool(name="ps", bufs=1, space="PSUM") as psp,
        ):
            ewt = sb.tile([P, BT], f32, name="ewt", tag="ewt")
            dmy = sb.tile([P, 512], bf16, name="dmy", tag="dmy")
            scr_v = sb.tile([1, BT], f32, name="scrv", tag="scrv")
            scr_s = sb.tile([1, BT], f32, name="scrs", tag="scrs")
            wxs = [
                sb.tile([P, (e - s) * CW], bf16, name=f"wx{ci}", tag=f"wx{ci}")
                for ci, (s, e) in enumerate(CHUNKS)
            ]
            y_v = sb.tile([P, BT * 512], bf16, name="yv", tag="yv")
            y_a = sb.tile([P, BT * 512], bf16, name="ya", tag="ya")
            pss = [
                [
                    psp.tile([P, 512], f32, name=f"ps{t}{h}", tag=f"ps{t}{h}")
                    for h in range(NH)
                ]
                for t in range(BT)
            ]

            # exactly 8 HWDGE DMAs in the whole kernel -> each DMAHW lane
            # is used once, so no DMA ever needs a lane-recycle wait on
            # top of its data wait (single-wait limit).
            nc.sync.dma_start(ewt[:], ew_d[:])
            for ci, (s, e) in enumerate(CHUNKS):
                src = wx_d[s * P:e * P, :].rearrange("(n p) c -> p n c", p=P)
                dst = wxs[ci][:].rearrange("p (n c) -> p n c", n=e - s)
                nc.sync.dma_start(dst, src)

            # HAM warmers: zero matmuls during the DMA lead-in
            nc.vector.memset(dmy[:], 0.0)
            for _ in range(N_DUMMY):
                nc.tensor.matmul(
                    pss[0][0][0:1, :], dmy[:, 0:1], dmy[:, :],
                    start=True, stop=True, skip_group_check=True,
                )

            # i-major accumulation: group n waits only on its chunk's DMA
            for n in range(NIT):
                ci, (s, e) = next(
                    (i, c) for i, c in enumerate(CHUNKS) if c[0] <= n < c[1]
                )
                wx = wxs[ci]
                off = (n - s) * CW
                for t in range(BT):
                    lhsT = wx[:, off + XOFF + P * t:off + XOFF + P * (t + 1)]
                    for h in range(NH):
                        nc.tensor.matmul(
                            pss[t][h][:], lhsT,
                            wx[:, off + 512 * h:off + 512 * (h + 1)],
                            start=(n == 0), stop=(n == NIT - 1),
                            skip_group_check=(t == 0 and h == 0),
                        )

            # absorb the ewt DMA wait on each evict engine so the real
            # evictions carry only their PE wait (single-wait limit)
            nc.vector.tensor_copy(scr_v[:], ewt[0:1, :])
            nc.scalar.activation(scr_s[:], ewt[0:1, :], Copy)

            # evict: y[b,:] = ps[b,:] * ew[b]; DVE takes h=0, ACT h=1
            for t in range(BT):
                sc = ewt[:, t:t + 1]
                nc.vector.tensor_scalar_mul(
                    y_v[:, t * 512:(t + 1) * 512], pss[t][0][:], sc
                )
                nc.scalar.activation(
                    y_a[:, t * 512:(t + 1) * 512], pss[t][1][:], Copy, scale=sc
                )
            nc.sync.dma_start(yv_d[:], y_v[:])
            nc.sync.dma_start(ya_d[:], y_a[:])

    return nc


def _get_compiled():
    global _compiled
    if _compiled is None:
        _compiled = _build()
    return _compiled


_pack_cache = None


def _make_in_maps(x, expert_weights, weight, bias):
    global _pack_cache
    import ml_dtypes

    bf16 = ml_dtypes.bfloat16
    if _pack_cache is None or _pack_cache[0] is not weight:
        w = np.asarray(weight, dtype=np.float32)
        wxs = []
        for c in range(NCORES):
            a = np.zeros((NIT, P, CW), dtype=bf16)
            # wT tile n: [p, o] = W[c, o, 128n+p]
            a[:, :, :OUT] = w[c].T.reshape(NIT, P, OUT).astype(bf16)
            wxs.append(a)
        _pack_cache = (weight, wxs)
    wxs = _pack_cache[1]

    x = np.asarray(x, dtype=np.float32)
    ew = np.asarray(expert_weights, dtype=np.float32)
    # xT tile n: [p, b] = x[b, 128n+p]
    xTb = x.T.reshape(NIT, P, B).astype(bf16)
    in_maps = []
    for c in range(NCORES):
        wxs[c][:, :, XOFF:] = xTb
        ewt = np.ascontiguousarray(ew[:, c].reshape(BT, P).T)
        in_maps.append({
            "wx": wxs[c].reshape(NIT * P, CW),
            "ew": ewt,
        })
    return in_maps


def kernel(x, expert_weights, weight, bias, _trace=False):
    from concourse.bass_utils import run_bass_kernel_spmd

    nc = _get_compiled()
    in_maps = _make_in_maps(x, expert_weights, weight, bias)
    res = run_bass_kernel_spmd(
        nc, in_maps, core_ids=list(range(NCORES)), trace=_trace
    )
    acc = np.zeros((B, OUT), dtype=np.float32)
    for r in res.results:
        # yv[p, t*512+j] = y[128t+p, j]; ya[p, t*512+j] = y[128t+p, 512+j]
        yv = np.asarray(r["yv"], dtype=np.float32).reshape(P, BT, 512)
        ya = np.asarray(r["ya"], dtype=np.float32).reshape(P, BT, 512)
        acc[:, :512] += yv.transpose(1, 0, 2).reshape(B, 512)
        acc[:, 512:] += ya.transpose(1, 0, 2).reshape(B, 512)
    ew = np.asarray(expert_weights, dtype=np.float32)
    b = np.asarray(bias, dtype=np.float32)
    y = acc + ew @ b
    if _trace:
        return y, res
    return y


# revision 15
# speedup vs baseline: 2.3275x; 1.0581x over previous
"""ExpertLinear (dense MoE blend) Trainium2 kernel — expert-sharded.

y[b,o] = sum_k ew[b,k] * (x[b,:] @ W[k,o,:]) + sum_k ew[b,k] * bias[k,o]

Sharding: one expert per core (E == 8 == NCORES). Each core computes its
expert's full GEMM z_c = x @ W[c].T for ALL B rows, scales by ew[:, c] on
eviction, and writes a bf16 partial; the host sums the 8 partials and adds
the (tiny) bias term. This reads each expert's weights exactly once
chip-wide: per-core HBM traffic is ~4 MB (vs ~18.5 MB for data-parallel),
turning a DMA-bound kernel into a PE-bound one (~13.7 us of bf16 matmul).

Layout/precision:
  - Host packs, per core, an interleaved stream of 8 i-chunks; chunk n =
    [wT tile n | xT tile n] as one [128, 1536] bf16 DMA, so the PE's
    i-major loop starts after one chunk lands and each matmul group needs
    exactly ONE sync wait (this walrus build rejects >1 wait/instruction).
  - ew column arrives fp32 [128, 4]; scaling happens on eviction via
    per-partition tensor_scalar_mul (DVE) / activation Copy scale (ACT),
    split across both engines so the eviction tail halves.
  - PSUM: all 8 banks hold the [512, 1024] fp32 partial (4 b-chunks x 2
    o-halves); i-major accumulation, single eviction per bank.
  - A few zero-matmuls at the start keep the PE busy during the DMA
    lead-in so the HAM clock-gate un-throttles (1.2 -> 2.4 GHz) sooner.
"""

import numpy as np

B, E, IN, OUT = 512, 8, 1024, 1024
NCORES = 8
P = 128
NIT = IN // P      # 8 i-tiles (contraction chunks)
BT = B // P        # 4 b-chunks (output partition tiles)
NH = OUT // 512    # 2 o-halves (PSUM bank free-dim limit)
CW = OUT + B  # 1536 cols per i-tile: wT tile (1024) + xT tile (512)
XOFF = OUT          # x region offset inside an i-tile block
N_DUMMY = 14
# i-tile ranges per DMA chunk: small first chunks let the PE start early,
# and 5 input chunks + ew + 2 outputs = 8 DMAs = one per DMAHW lane
CHUNKS = [(0, 1), (1, 2), (2, 4), (4, 6), (6, 8)]

_compiled = None


def _patch_drain_split():
    """The walrus build in this container rejects any instruction carrying
    more than one sync wait, including the kernel-tail Drain that
    TileContext emits with one wait per active semaphore. Split it into a
    sequence of single-wait drains (sequencer-FIFO keeps them ordered;
    the set of waits is identical)."""
    import concourse.tile as tile_mod

    if getattr(tile_mod.TileContext, "_drain_split_patched", False):
        return
    from concourse.tile_sem_assignment import N_PROCS
    from concourse.vector_clock import ScopedClock, VectorClock

    def _drain_and_barrier(self, tick_clock, wait_clock):
        gc = tick_clock.global_clock
        for p in range(N_PROCS):
            t = gc[p]
            if t <= 0:
                continue
            ticks = [0] * N_PROCS
            ticks[p] = t
            di = self.nc.sync.drain()
            wait_clock.add_sem_waits(
                di.ins, ScopedClock({None: VectorClock(ticks)})
            )
        self.nc.all_engine_barrier()
        assert self.sems is not None
        popped = self.nc._tile_sem_poison_stack.pop()
        assert popped is self._sem_poison
        self.nc.clear_and_free_semaphores(list(self.sems.allocated().values()))
        self.nc.all_engine_barrier()

    tile_mod.TileContext._drain_and_barrier = _drain_and_barrier
    tile_mod.TileContext._drain_split_patched = True


def _build():
    import concourse.bass as bass
    import concourse.mybir as mybir
    import concourse.tile as tile

    _patch_drain_split()

    f32 = mybir.dt.float32
    bf16 = mybir.dt.bfloat16
    Copy = mybir.ActivationFunctionType.Copy

    nc = bass.Bass()
    wx_d = nc.dram_tensor("wx", [NIT * P, CW], bf16, kind="ExternalInput")
    ew_d = nc.dram_tensor("ew", [P, BT], f32, kind="ExternalInput")
    yv_d = nc.dram_tensor("yv", [P, BT * 512], bf16, kind="ExternalOutput")
    ya_d = nc.dram_tensor("ya", [P, BT * 512], bf16, kind="ExternalOutput")

    with tile.TileContext(nc) as tc:
        with (
            tc.tile_pool(name="sb", bufs=1) as sb,
            tc.tile_pool(name="ps", bufs=1, space="PSUM") as psp,
        ):
            ewt = sb.tile([P, BT], f32, name="ewt", tag="ewt")
            scr_v = sb.tile([1, BT], f32, name="scrv", tag="scrv")
            scr_s = sb.tile([1, BT], f32, name="scrs", tag="scrs")
            wxs = [
                sb.tile([P, (e - s) * CW], bf16, name=f"wx{ci}", tag=f"wx{ci}")
                for ci, (s, e) in enumerate(CHUNKS)
            ]
            y_v = sb.tile([P, BT * 512], bf16, name="yv", tag="yv")
            y_a = sb.tile([P, BT * 512], bf16, name="ya", tag="ya")
            pss = [
                [
                    psp.tile([P, 512], f32, name=f"ps{t}{h}", tag=f"ps{t}{h}")
                    for h in range(NH)
                ]
                for t in range(BT)
            ]

            # HAM warmers: matmuls over (uninitialized) y_v keep the PE
            # array busy from engine-boot until the first chunk lands, so
            # the clock-gate reaches 8/8 before the real matmuls start.
            # Their garbage output lands in bank (0,0), which the real
            # group's start=True clears.
            for _ in range(N_DUMMY):
                nc.tensor.matmul(
                    pss[0][0][0:1, :], y_v[:, 0:1], y_v[:, 0:512],
                    start=True, stop=True, skip_group_check=True,
                )

            # exactly 8 HWDGE DMAs in the whole kernel -> each DMAHW lane
            # is used once, so no DMA ever needs a lane-recycle wait on
            # top of its data wait (single-wait limit). wx0 first so the
            # PE's first real group is gated only by it; ew is not needed
            # until eviction.
            for ci, (s, e) in enumerate(CHUNKS):
                src = wx_d[s * P:e * P, :].rearrange("(n p) c -> p n c", p=P)
                dst = wxs[ci][:].rearrange("p (n c) -> p n c", n=e - s)
                nc.sync.dma_start(dst, src)
            nc.sync.dma_start(ewt[:], ew_d[:])

            # accumulation: chunk-major so group n waits only on its
            # chunk's DMA; within a chunk, bank-major (t, h, n) so banks
            # finish staggered in the last chunk and evictions pipeline
            # behind the PE instead of serializing after it.
            for ci, (s, e) in enumerate(CHUNKS):
                wx = wxs[ci]
                for t in range(BT):
                    for n in range(s, e):
                        off = (n - s) * CW
                        lhsT = wx[
                            :, off + XOFF + P * t:off + XOFF + P * (t + 1)
                        ]
                        for h in range(NH):
                            nc.tensor.matmul(
                                pss[t][h][:], lhsT,
                                wx[:, off + 512 * h:off + 512 * (h + 1)],
                                start=(n == s and ci == 0),
                                stop=(n == e - 1 and ci == len(CHUNKS) - 1),
                                skip_group_check=(t == 0 and h == 0),
                            )

            # absorb the ewt DMA wait on each evict engine so the real
            # evictions carry only their PE wait (single-wait limit)
            nc.vector.tensor_copy(scr_v[:], ewt[0:1, :])
            nc.scalar.activation(scr_s[:], ewt[0:1, :], Copy)

            # evict: y[b,:] = ps[b,:] * ew[b]; DVE takes h=0, ACT h=1
            for t in range(BT):
                sc = ewt[:, t:t + 1]
                nc.vector.tensor_scalar_mul(
                    y_v[:, t * 512:(t + 1) * 512], pss[t][0][:], sc
                )
                nc.scalar.activation(
                    y_a[:, t * 512:(t + 1) * 512], pss[t][1][:], Copy, scale=sc
                )
            nc.sync.dma_start(yv_d[:], y_v[:])
            nc.sync.dma_start(ya_d[:], y_a[:])

    return nc


def _get_compiled():
    global _compiled
    if _compiled is None:
        _compiled = _build()
    return _compiled


_pack_cache = None


def _make_in_maps(x, expert_weights, weight, bias):
    global _pack_cache
    import ml_dtypes

    bf16 = ml_dtypes.bfloat16
    if _pack_cache is None or _pack_cache[0] is not weight:
        w = np.asarray(weight, dtype=np.float32)
        wxs = []
        for c in range(NCORES):
            a = np.zeros((NIT, P, CW), dtype=bf16)
            # wT tile n: [p, o] = W[c, o, 128n+p]
            a[:, :, :OUT] = w[c].T.reshape(NIT, P, OUT).astype(bf16)
            wxs.append(a)
        _pack_cache = (weight, wxs)
    wxs = _pack_cache[1]

    x = np.asarray(x, dtype=np.float32)
    ew = np.asarray(expert_weights, dtype=np.float32)
    # xT tile n: [p, b] = x[b, 128n+p]
    xTb = x.T.reshape(NIT, P, B).astype(bf16)
    in_maps = []
    for c in range(NCORES):
        wxs[c][:, :, XOFF:] = xTb
        ewt = np.ascontiguousarray(ew[:, c].reshape(BT, P).T)
        in_maps.append({
            "wx": wxs[c].reshape(NIT * P, CW),
            "ew": ewt,
        })
    return in_maps


def kernel(x, expert_weights, weight, bias, _trace=False):
    from concourse.bass_utils import run_bass_kernel_spmd

    nc = _get_compiled()
    in_maps = _make_in_maps(x, expert_weights, weight, bias)
    res = run_bass_kernel_spmd(
        nc, in_maps, core_ids=list(range(NCORES)), trace=_trace
    )
    acc = np.zeros((B, OUT), dtype=np.float32)
    for r in res.results:
        # yv[p, t*512+j] = y[128t+p, j]; ya[p, t*512+j] = y[128t+p, 512+j]
        yv = np.asarray(r["yv"], dtype=np.float32).reshape(P, BT, 512)
        ya = np.asarray(r["ya"], dtype=np.float32).reshape(P, BT, 512)
        acc[:, :512] += yv.transpose(1, 0, 2).reshape(B, 512)
        acc[:, 512:] += ya.transpose(1, 0, 2).reshape(B, 512)
    ew = np.asarray(expert_weights, dtype=np.float32)
    b = np.asarray(bias, dtype=np.float32)
    y = acc + ew @ b
    if _trace:
        return y, res
    return y


# revision 18
# speedup vs baseline: 2.3793x; 1.0222x over previous
"""ExpertLinear (dense MoE blend) Trainium2 kernel — expert-sharded.

y[b,o] = sum_k ew[b,k] * (x[b,:] @ W[k,o,:]) + sum_k ew[b,k] * bias[k,o]

Sharding: one expert per core (E == 8 == NCORES). Each core computes its
expert's full GEMM z_c = x @ W[c].T for ALL B rows, scales by ew[:, c] on
eviction, and writes a bf16 partial; the host sums the 8 partials and adds
the (tiny) bias term. This reads each expert's weights exactly once
chip-wide: per-core HBM traffic is ~4 MB (vs ~18.5 MB for data-parallel),
turning a DMA-bound kernel into a PE-bound one (~13.7 us of bf16 matmul).

Layout/precision:
  - Host packs, per core, an interleaved stream of 8 i-chunks; chunk n =
    [wT tile n | xT tile n] as one [128, 1536] bf16 DMA, so the PE's
    i-major loop starts after one chunk lands and each matmul group needs
    exactly ONE sync wait (this walrus build rejects >1 wait/instruction).
  - ew column arrives fp32 [128, 4]; scaling happens on eviction via
    per-partition tensor_scalar_mul (DVE) / activation Copy scale (ACT),
    split across both engines so the eviction tail halves.
  - PSUM: all 8 banks hold the [512, 1024] fp32 partial (4 b-chunks x 2
    o-halves); i-major accumulation, single eviction per bank.
  - A few zero-matmuls at the start keep the PE busy during the DMA
    lead-in so the HAM clock-gate un-throttles (1.2 -> 2.4 GHz) sooner.
"""

import numpy as np

B, E, IN, OUT = 512, 8, 1024, 1024
NCORES = 8
P = 128
NIT = IN // P      # 8 i-tiles (contraction chunks)
BT = B // P        # 4 b-chunks (output partition tiles)
NH = OUT // 512    # 2 o-halves (PSUM bank free-dim limit)
CW = OUT + B  # 1536 cols per i-tile: wT tile (1024) + xT tile (512)
XOFF = OUT          # x region offset inside an i-tile block
N_DUMMY = 13
EWPAD = 16          # extra bf16 cols on chunk 0 carrying the ew column
# i-tile ranges per DMA chunk: small first chunks let the PE start early,
# and 5 input chunks + 3 outputs = 8 DMAs = one per DMAHW lane
CHUNKS = [(0, 1), (1, 2), (2, 4), (4, 6), (6, 8)]

_compiled = None


def _patch_drain_split():
    """The walrus build in this container rejects any instruction carrying
    more than one sync wait, including the kernel-tail Drain that
    TileContext emits with one wait per active semaphore. Split it into a
    sequence of single-wait drains (sequencer-FIFO keeps them ordered;
    the set of waits is identical)."""
    import concourse.tile as tile_mod

    if getattr(tile_mod.TileContext, "_drain_split_patched", False):
        return
    from concourse.tile_sem_assignment import N_PROCS
    from concourse.vector_clock import ScopedClock, VectorClock

    def _drain_and_barrier(self, tick_clock, wait_clock):
        gc = tick_clock.global_clock
        for p in range(N_PROCS):
            t = gc[p]
            if t <= 0:
                continue
            ticks = [0] * N_PROCS
            ticks[p] = t
            di = self.nc.sync.drain()
            wait_clock.add_sem_waits(
                di.ins, ScopedClock({None: VectorClock(ticks)})
            )
        self.nc.all_engine_barrier()
        assert self.sems is not None
        popped = self.nc._tile_sem_poison_stack.pop()
        assert popped is self._sem_poison
        self.nc.clear_and_free_semaphores(list(self.sems.allocated().values()))
        self.nc.all_engine_barrier()

    tile_mod.TileContext._drain_and_barrier = _drain_and_barrier
    tile_mod.TileContext._drain_split_patched = True


def _build():
    import concourse.bass as bass
    import concourse.mybir as mybir
    import concourse.tile as tile

    _patch_drain_split()

    f32 = mybir.dt.float32
    bf16 = mybir.dt.bfloat16
    Copy = mybir.ActivationFunctionType.Copy

    nc = bass.Bass()
    wx0_d = nc.dram_tensor("wx0", [P, CW + EWPAD], bf16, kind="ExternalInput")
    wxr_d = nc.dram_tensor(
        "wxr", [(NIT - 1) * P, CW], bf16, kind="ExternalInput"
    )
    yv_d = nc.dram_tensor("yv", [P, BT * 512], bf16, kind="ExternalOutput")
    ya_d = nc.dram_tensor("ya", [P, BT * 512], bf16, kind="ExternalOutput")

    with tile.TileContext(nc) as tc:
        with (
            tc.tile_pool(name="sb", bufs=1) as sb,
            tc.tile_pool(name="ps", bufs=1, space="PSUM") as psp,
        ):
            ewt = sb.tile([P, BT], f32, name="ewt", tag="ewt")
            scr_v = sb.tile([P, 1], f32, name="scrv", tag="scrv")
            scr_s = sb.tile([1, BT], f32, name="scrs", tag="scrs")
            wxs = [
                sb.tile(
                    [P, (e - s) * CW + (EWPAD if ci == 0 else 0)],
                    bf16, name=f"wx{ci}", tag=f"wx{ci}",
                )
                for ci, (s, e) in enumerate(CHUNKS)
            ]
            y_v = sb.tile([P, BT * 512], bf16, name="yv", tag="yv")
            y_a = sb.tile([P, BT * 512], bf16, name="ya", tag="ya")
            pss = [
                [
                    psp.tile([P, 512], f32, name=f"ps{t}{h}", tag=f"ps{t}{h}")
                    for h in range(NH)
                ]
                for t in range(BT)
            ]

            # HAM warmers: matmuls over (uninitialized) y_v keep the PE
            # array busy from engine-boot until the first chunk lands, so
            # the clock-gate reaches 8/8 before the real matmuls start.
            # Their garbage output lands in bank (0,0), which the real
            # group's start=True clears.
            for _ in range(N_DUMMY):
                nc.tensor.matmul(
                    pss[0][0][0:1, :], y_v[:, 0:1], y_v[:, 0:512],
                    start=True, stop=True, skip_group_check=True,
                )

            # exactly 8 HWDGE DMAs in the whole kernel -> each DMAHW lane
            # is used once, so no DMA ever needs a lane-recycle wait on
            # top of its data wait (single-wait limit). wx0 first so the
            # PE's first real group is gated only by it; ew is not needed
            # until eviction.
            nc.sync.dma_start(wxs[0][:], wx0_d[:])
            for ci, (s, e) in enumerate(CHUNKS[1:], start=1):
                src = wxr_d[(s - 1) * P:(e - 1) * P, :].rearrange(
                    "(n p) c -> p n c", p=P
                )
                dst = wxs[ci][:].rearrange("p (n c) -> p n c", n=e - s)
                nc.sync.dma_start(dst, src)

            # accumulation: chunk-major so group n waits only on its
            # chunk's DMA; within a chunk, bank-major (t, h, n) so banks
            # finish staggered in the last chunk and evictions pipeline
            # behind the PE instead of serializing after it.
            for ci, (s, e) in enumerate(CHUNKS):
                wx = wxs[ci]
                for t in range(BT):
                    for n in range(s, e):
                        off = (n - s) * CW
                        lhsT = wx[
                            :, off + XOFF + P * t:off + XOFF + P * (t + 1)
                        ]
                        for h in range(NH):
                            nc.tensor.matmul(
                                pss[t][h][:], lhsT,
                                wx[:, off + 512 * h:off + 512 * (h + 1)],
                                start=(n == s and ci == 0),
                                stop=(n == e - 1 and ci == len(CHUNKS) - 1),
                                skip_group_check=(t == 0 and h == 0),
                            )

            # ew rides in chunk 0 as bf16; DVE upconverts it once (this
            # also absorbs the chunk-0 DMA wait for DVE), and the ACT
            # absorber reads the converted copy so real evictions carry
            # only their PE wait (single-wait limit)
            nc.vector.tensor_copy(ewt[:], wxs[0][:, CW:CW + BT])
            # absorber: reads ewt through the tensor_scalar ptr path so the
            # real DVE evicts don't carry a second (DVE-seq) wait
            nc.vector.tensor_scalar_mul(scr_v[:], wxs[0][:, 0:1], ewt[:, 0:1])
            nc.scalar.activation(scr_s[:], ewt[0:1, :], Copy)

            # evict: y[b,:] = ps[b,:] * ew[b]; DVE takes h=0, ACT h=1
            for t in range(BT):
                sc = ewt[:, t:t + 1]
                nc.vector.tensor_scalar_mul(
                    y_v[:, t * 512:(t + 1) * 512], pss[t][0][:], sc
                )
                nc.scalar.activation(
                    y_a[:, t * 512:(t + 1) * 512], pss[t][1][:], Copy, scale=sc
                )
            nc.sync.dma_start(ya_d[:, 0:1024], y_a[:, 0:1024])
            nc.sync.dma_start(yv_d[:], y_v[:])
            nc.sync.dma_start(ya_d[:, 1024:2048], y_a[:, 1024:2048])

    return nc


def _get_compiled():
    global _compiled
    if _compiled is None:
        _compiled = _build()
    return _compiled


_pack_cache = None


def _make_in_maps(x, expert_weights, weight, bias):
    global _pack_cache
    import ml_dtypes

    bf16 = ml_dtypes.bfloat16
    if _pack_cache is None or _pack_cache[0] is not weight:
        w = np.asarray(weight, dtype=np.float32)
        wx0s, wxrs = [], []
        for c in range(NCORES):
            wT = w[c].T.reshape(NIT, P, OUT).astype(bf16)  # [p,o]=W[c,o,128n+p]
            a0 = np.zeros((P, CW + EWPAD), dtype=bf16)
            a0[:, :OUT] = wT[0]
            ar = np.zeros((NIT - 1, P, CW), dtype=bf16)
            ar[:, :, :OUT] = wT[1:]
            wx0s.append(a0)
            wxrs.append(ar)
        _pack_cache = (weight, wx0s, wxrs)
    _, wx0s, wxrs = _pack_cache

    x = np.asarray(x, dtype=np.float32)
    ew = np.asarray(expert_weights, dtype=np.float32)
    # xT tile n: [p, b] = x[b, 128n+p]
    xTb = x.T.reshape(NIT, P, B).astype(bf16)
    in_maps = []
    for c in range(NCORES):
        wx0s[c][:, XOFF:XOFF + B] = xTb[0]
        wx0s[c][:, CW:CW + BT] = ew[:, c].reshape(BT, P).T.astype(bf16)
        wxrs[c][:, :, XOFF:] = xTb[1:]
        in_maps.append({
            "wx0": wx0s[c],
            "wxr": wxrs[c].reshape((NIT - 1) * P, CW),
        })
    return in_maps


def kernel(x, expert_weights, weight, bias, _trace=False):
    from concourse.bass_utils import run_bass_kernel_spmd

    nc = _get_compiled()
    in_maps = _make_in_maps(x, expert_weights, weight, bias)
    res = run_bass_kernel_spmd(
        nc, in_maps, core_ids=list(range(NCORES)), trace=_trace
    )
    acc = np.zeros((B, OUT), dtype=np.float32)
    for r in res.results:
        # yv[p, t*512+j] = y[128t+p, j]; ya[p, t*512+j] = y[128t+p, 512+j]
        yv = np.asarray(r["yv"], dtype=np.float32).reshape(P, BT, 512)
        ya = np.asarray(r["ya"], dtype=np.float32).reshape(P, BT, 512)
        acc[:, :512] += yv.transpose(1, 0, 2).reshape(B, 512)
        acc[:, 512:] += ya.transpose(1, 0, 2).reshape(B, 512)
    ew = np.asarray(expert_weights, dtype=np.float32)
    b = np.asarray(bias, dtype=np.float32)
    y = acc + ew @ b
    if _trace:
        return y, res
    return y


# revision 19
# speedup vs baseline: 2.4226x; 1.0182x over previous
"""ExpertLinear (dense MoE blend) Trainium2 kernel — expert-sharded.

y[b,o] = sum_k ew[b,k] * (x[b,:] @ W[k,o,:]) + sum_k ew[b,k] * bias[k,o]

Sharding: one expert per core (E == 8 == NCORES). Each core computes its
expert's full GEMM z_c = x @ W[c].T for ALL B rows, scales by ew[:, c] on
eviction, and writes a bf16 partial; the host sums the 8 partials and adds
the (tiny) bias term. This reads each expert's weights exactly once
chip-wide: per-core HBM traffic is ~4 MB (vs ~18.5 MB for data-parallel),
turning a DMA-bound kernel into a PE-bound one (~13.7 us of bf16 matmul).

Layout/precision:
  - Host packs, per core, an interleaved stream of 8 i-chunks; chunk n =
    [wT tile n | xT tile n] as one [128, 1536] bf16 DMA, so the PE's
    i-major loop starts after one chunk lands and each matmul group needs
    exactly ONE sync wait (this walrus build rejects >1 wait/instruction).
  - ew column arrives fp32 [128, 4]; scaling happens on eviction via
    per-partition tensor_scalar_mul (DVE) / activation Copy scale (ACT),
    split across both engines so the eviction tail halves.
  - PSUM: all 8 banks hold the [512, 1024] fp32 partial (4 b-chunks x 2
    o-halves); i-major accumulation, single eviction per bank.
  - A few zero-matmuls at the start keep the PE busy during the DMA
    lead-in so the HAM clock-gate un-throttles (1.2 -> 2.4 GHz) sooner.
"""

import numpy as np

B, E, IN, OUT = 512, 8, 1024, 1024
NCORES = 8
P = 128
NIT = IN // P      # 8 i-tiles (contraction chunks)
BT = B // P        # 4 b-chunks (output partition tiles)
NH = OUT // 512    # 2 o-halves (PSUM bank free-dim limit)
CW = OUT + B  # 1536 cols per i-tile: wT tile (1024) + xT tile (512)
XOFF = OUT          # x region offset inside an i-tile block
N_DUMMY = 13
EWPAD = 16          # extra bf16 cols on chunk 0 carrying the ew column
# i-tile ranges per DMA chunk: a small first chunk lets the PE start early,
# and 4 input chunks + 4 outputs = 8 DMAs = one per DMAHW lane
CHUNKS = [(0, 1), (1, 3), (3, 5), (5, 8)]

_compiled = None


def _patch_drain_split():
    """The walrus build in this container rejects any instruction carrying
    more than one sync wait, including the kernel-tail Drain that
    TileContext emits with one wait per active semaphore. Split it into a
    sequence of single-wait drains (sequencer-FIFO keeps them ordered;
    the set of waits is identical)."""
    import concourse.tile as tile_mod

    if getattr(tile_mod.TileContext, "_drain_split_patched", False):
        return
    from concourse.tile_sem_assignment import N_PROCS
    from concourse.vector_clock import ScopedClock, VectorClock

    def _drain_and_barrier(self, tick_clock, wait_clock):
        gc = tick_clock.global_clock
        for p in range(N_PROCS):
            t = gc[p]
            if t <= 0:
                continue
            ticks = [0] * N_PROCS
            ticks[p] = t
            di = self.nc.sync.drain()
            wait_clock.add_sem_waits(
                di.ins, ScopedClock({None: VectorClock(ticks)})
            )
        self.nc.all_engine_barrier()
        assert self.sems is not None
        popped = self.nc._tile_sem_poison_stack.pop()
        assert popped is self._sem_poison
        # bookkeeping of clear_and_free_semaphores WITHOUT emitting the
        # gpsimd clear + trailing barrier: the NEFF-level teardown wipes
        # the whole sem space anyway, and nothing in this program runs
        # after the barrier above -- saves ~1 us of kernel tail
        sem_nums = [s.num for s in self.sems.allocated().values()]
        self.nc._state.prepend_free_semaphores(sem_nums)
        for poison_set in self.nc._tile_sem_poison_stack:
            poison_set.update(sem_nums)

    tile_mod.TileContext._drain_and_barrier = _drain_and_barrier
    tile_mod.TileContext._drain_split_patched = True


def _build():
    import concourse.bass as bass
    import concourse.mybir as mybir
    import concourse.tile as tile

    _patch_drain_split()

    f32 = mybir.dt.float32
    bf16 = mybir.dt.bfloat16
    Copy = mybir.ActivationFunctionType.Copy

    nc = bass.Bass()
    wx0_d = nc.dram_tensor("wx0", [P, CW + EWPAD], bf16, kind="ExternalInput")
    wxr_d = nc.dram_tensor(
        "wxr", [(NIT - 1) * P, CW], bf16, kind="ExternalInput"
    )
    yv_d = nc.dram_tensor("yv", [P, BT * 512], bf16, kind="ExternalOutput")
    ya_d = nc.dram_tensor("ya", [P, BT * 512], bf16, kind="ExternalOutput")

    with tile.TileContext(nc) as tc:
        with (
            tc.tile_pool(name="sb", bufs=1) as sb,
            tc.tile_pool(name="ps", bufs=1, space="PSUM") as psp,
        ):
            ewt = sb.tile([P, BT], f32, name="ewt", tag="ewt")
            scr_v = sb.tile([P, 1], f32, name="scrv", tag="scrv")
            scr_s = sb.tile([1, BT], f32, name="scrs", tag="scrs")
            wxs = [
                sb.tile(
                    [P, (e - s) * CW + (EWPAD if ci == 0 else 0)],
                    bf16, name=f"wx{ci}", tag=f"wx{ci}",
                )
                for ci, (s, e) in enumerate(CHUNKS)
            ]
            y_v = sb.tile([P, BT * 512], bf16, name="yv", tag="yv")
            y_a = sb.tile([P, BT * 512], bf16, name="ya", tag="ya")
            pss = [
                [
                    psp.tile([P, 512], f32, name=f"ps{t}{h}", tag=f"ps{t}{h}")
                    for h in range(NH)
                ]
                for t in range(BT)
            ]

            # HAM warmers: matmuls over (uninitialized) y_v keep the PE
            # array busy from engine-boot until the first chunk lands, so
            # the clock-gate reaches 8/8 before the real matmuls start.
            # Their garbage output lands in bank (0,0), which the real
            # group's start=True clears.
            for _ in range(N_DUMMY):
                nc.tensor.matmul(
                    pss[0][0][0:1, :], y_v[:, 0:1], y_v[:, 0:512],
                    start=True, stop=True, skip_group_check=True,
                )

            # exactly 8 HWDGE DMAs in the whole kernel -> each DMAHW lane
            # is used once, so no DMA ever needs a lane-recycle wait on
            # top of its data wait (single-wait limit). wx0 first so the
            # PE's first real group is gated only by it; ew is not needed
            # until eviction.
            nc.sync.dma_start(wxs[0][:], wx0_d[:])
            for ci, (s, e) in enumerate(CHUNKS[1:], start=1):
                src = wxr_d[(s - 1) * P:(e - 1) * P, :].rearrange(
                    "(n p) c -> p n c", p=P
                )
                dst = wxs[ci][:].rearrange("p (n c) -> p n c", n=e - s)
                nc.sync.dma_start(dst, src)

            # accumulation: chunk-major so group n waits only on its
            # chunk's DMA; within a chunk, bank-major (t, h, n) so banks
            # finish staggered in the last chunk and evictions pipeline
            # behind the PE instead of serializing after it.
            for ci, (s, e) in enumerate(CHUNKS):
                wx = wxs[ci]
                for t in range(BT):
                    for n in range(s, e):
                        off = (n - s) * CW
                        lhsT = wx[
                            :, off + XOFF + P * t:off + XOFF + P * (t + 1)
                        ]
                        for h in range(NH):
                            nc.tensor.matmul(
                                pss[t][h][:], lhsT,
                                wx[:, off + 512 * h:off + 512 * (h + 1)],
                                start=(n == s and ci == 0),
                                stop=(n == e - 1 and ci == len(CHUNKS) - 1),
                                skip_group_check=(t == 0 and h == 0),
                            )

            # ew rides in chunk 0 as bf16; DVE upconverts it once (this
            # also absorbs the chunk-0 DMA wait for DVE), and the ACT
            # absorber reads the converted copy so real evictions carry
            # only their PE wait (single-wait limit)
            nc.vector.tensor_copy(ewt[:], wxs[0][:, CW:CW + BT])
            # absorber: reads ewt through the tensor_scalar ptr path so the
            # real DVE evicts don't carry a second (DVE-seq) wait
            nc.vector.tensor_scalar_mul(scr_v[:], wxs[0][:, 0:1], ewt[:, 0:1])
            nc.scalar.activation(scr_s[:], ewt[0:1, :], Copy)

            # evict: y[b,:] = ps[b,:] * ew[b]; DVE takes h=0, ACT h=1
            for t in range(BT):
                sc = ewt[:, t:t + 1]
                nc.vector.tensor_scalar_mul(
                    y_v[:, t * 512:(t + 1) * 512], pss[t][0][:], sc
                )
                nc.scalar.activation(
                    y_a[:, t * 512:(t + 1) * 512], pss[t][1][:], Copy, scale=sc
                )
            nc.sync.dma_start(yv_d[:, 0:1024], y_v[:, 0:1024])
            nc.sync.dma_start(ya_d[:, 0:1024], y_a[:, 0:1024])
            nc.sync.dma_start(yv_d[:, 1024:2048], y_v[:, 1024:2048])
            nc.sync.dma_start(ya_d[:, 1024:2048], y_a[:, 1024:2048])

    return nc


def _get_compiled():
    global _compiled
    if _compiled is None:
        _compiled = _build()
    return _compiled


_pack_cache = None


def _make_in_maps(x, expert_weights, weight, bias):
    global _pack_cache
    import ml_dtypes

    bf16 = ml_dtypes.bfloat16
    if _pack_cache is None or _pack_cache[0] is not weight:
        w = np.asarray(weight, dtype=np.float32)
        wx0s, wxrs = [], []
        for c in range(NCORES):
            wT = w[c].T.reshape(NIT, P, OUT).astype(bf16)  # [p,o]=W[c,o,128n+p]
            a0 = np.zeros((P, CW + EWPAD), dtype=bf16)
            a0[:, :OUT] = wT[0]
            ar = np.zeros((NIT - 1, P, CW), dtype=bf16)
            ar[:, :, :OUT] = wT[1:]
            wx0s.append(a0)
            wxrs.append(ar)
        _pack_cache = (weight, wx0s, wxrs)
    _, wx0s, wxrs = _pack_cache

    x = np.asarray(x, dtype=np.float32)
    ew = np.asarray(expert_weights, dtype=np.float32)
    # xT tile n: [p, b] = x[b, 128n+p]
    xTb = x.T.reshape(NIT, P, B).astype(bf16)
    in_maps = []
    for c in range(NCORES):
        wx0s[c][:, XOFF:XOFF + B] = xTb[0]
        wx0s[c][:, CW:CW + BT] = ew[:, c].reshape(BT, P).T.astype(bf16)
        wxrs[c][:, :, XOFF:] = xTb[1:]
        in_maps.append({
            "wx0": wx0s[c],
            "wxr": wxrs[c].reshape((NIT - 1) * P, CW),
        })
    return in_maps


def kernel(x, expert_weights, weight, bias, _trace=False):
    from concourse.bass_utils import run_bass_kernel_spmd

    nc = _get_compiled()
    in_maps = _make_in_maps(x, expert_weights, weight, bias)
    res = run_bass_kernel_spmd(
        nc, in_maps, core_ids=list(range(NCORES)), trace=_trace
    )
    acc = np.zeros((B, OUT), dtype=np.float32)
    for r in res.results:
        # yv[p, t*512+j] = y[128t+p, j]; ya[p, t*512+j] = y[128t+p, 512+j]
        yv = np.asarray(r["yv"], dtype=np.float32).reshape(P, BT, 512)
        ya = np.asarray(r["ya"], dtype=np.float32).reshape(P, BT, 512)
        acc[:, :512] += yv.transpose(1, 0, 2).reshape(B, 512)
        acc[:, 512:] += ya.transpose(1, 0, 2).reshape(B, 512)
    ew = np.asarray(expert_weights, dtype=np.float32)
    b = np.asarray(bias, dtype=np.float32)
    y = acc + ew @ b
    if _trace:
        return y, res
    return y


# revision 20
# speedup vs baseline: 2.4299x; 1.0030x over previous
"""ExpertLinear (dense MoE blend) Trainium2 kernel — expert-sharded.

y[b,o] = sum_k ew[b,k] * (x[b,:] @ W[k,o,:]) + sum_k ew[b,k] * bias[k,o]

Sharding: one expert per core (E == 8 == NCORES). Each core computes its
expert's full GEMM z_c = x @ W[c].T for ALL B rows, scales by ew[:, c] on
eviction, and writes a bf16 partial; the host sums the 8 partials and adds
the (tiny) bias term. This reads each expert's weights exactly once
chip-wide: per-core HBM traffic is ~4 MB (vs ~18.5 MB for data-parallel),
turning a DMA-bound kernel into a PE-bound one (~13.7 us of bf16 matmul).

Layout/precision:
  - Host packs, per core, an interleaved stream of 8 i-chunks; chunk n =
    [wT tile n | xT tile n] as one [128, 1536] bf16 DMA, so the PE's
    i-major loop starts after one chunk lands and each matmul group needs
    exactly ONE sync wait (this walrus build rejects >1 wait/instruction).
  - ew column arrives fp32 [128, 4]; scaling happens on eviction via
    per-partition tensor_scalar_mul (DVE) / activation Copy scale (ACT),
    split across both engines so the eviction tail halves.
  - PSUM: all 8 banks hold the [512, 1024] fp32 partial (4 b-chunks x 2
    o-halves); i-major accumulation, single eviction per bank.
  - A few zero-matmuls at the start keep the PE busy during the DMA
    lead-in so the HAM clock-gate un-throttles (1.2 -> 2.4 GHz) sooner.
"""

import numpy as np

B, E, IN, OUT = 512, 8, 1024, 1024
NCORES = 8
P = 128
NIT = IN // P      # 8 i-tiles (contraction chunks)
BT = B // P        # 4 b-chunks (output partition tiles)
NH = OUT // 512    # 2 o-halves (PSUM bank free-dim limit)
CW = OUT + B  # 1536 cols per i-tile: wT tile (1024) + xT tile (512)
XOFF = OUT          # x region offset inside an i-tile block
N_DUMMY = 9
EWPAD = 16          # extra bf16 cols on chunk 0 carrying the ew column
# i-tile ranges per DMA chunk: small first chunks let the PE start early,
# and 5 input chunks + 3 outputs = 8 DMAs = one per DMAHW lane
CHUNKS = [(0, 1), (1, 2), (2, 4), (4, 6), (6, 8)]

_compiled = None


def _patch_drain_split():
    """The walrus build in this container rejects any instruction carrying
    more than one sync wait, including the kernel-tail Drain that
    TileContext emits with one wait per active semaphore. Split it into a
    sequence of single-wait drains (sequencer-FIFO keeps them ordered;
    the set of waits is identical)."""
    import concourse.tile as tile_mod

    if getattr(tile_mod.TileContext, "_drain_split_patched", False):
        return
    from concourse.tile_sem_assignment import N_PROCS
    from concourse.vector_clock import ScopedClock, VectorClock

    def _drain_and_barrier(self, tick_clock, wait_clock):
        gc = tick_clock.global_clock
        for p in range(N_PROCS):
            t = gc[p]
            if t <= 0:
                continue
            ticks = [0] * N_PROCS
            ticks[p] = t
            di = self.nc.sync.drain()
            wait_clock.add_sem_waits(
                di.ins, ScopedClock({None: VectorClock(ticks)})
            )
        self.nc.all_engine_barrier()
        assert self.sems is not None
        popped = self.nc._tile_sem_poison_stack.pop()
        assert popped is self._sem_poison
        # bookkeeping of clear_and_free_semaphores WITHOUT emitting the
        # gpsimd clear + trailing barrier: the NEFF-level teardown wipes
        # the whole sem space anyway, and nothing in this program runs
        # after the barrier above -- saves ~1 us of kernel tail
        sem_nums = [s.num for s in self.sems.allocated().values()]
        self.nc._state.prepend_free_semaphores(sem_nums)
        for poison_set in self.nc._tile_sem_poison_stack:
            poison_set.update(sem_nums)

    tile_mod.TileContext._drain_and_barrier = _drain_and_barrier
    tile_mod.TileContext._drain_split_patched = True


def _build():
    import concourse.bass as bass
    import concourse.mybir as mybir
    import concourse.tile as tile

    _patch_drain_split()

    f32 = mybir.dt.float32
    bf16 = mybir.dt.bfloat16
    Copy = mybir.ActivationFunctionType.Copy

    nc = bass.Bass()
    wx0_d = nc.dram_tensor("wx0", [P, CW + EWPAD], bf16, kind="ExternalInput")
    wxr_d = nc.dram_tensor(
        "wxr", [(NIT - 1) * P, CW], bf16, kind="ExternalInput"
    )
    yv_d = nc.dram_tensor("yv", [P, BT * 512], bf16, kind="ExternalOutput")
    ya_d = nc.dram_tensor("ya", [P, BT * 512], bf16, kind="ExternalOutput")

    with tile.TileContext(nc) as tc:
        with (
            tc.tile_pool(name="sb", bufs=1) as sb,
            tc.tile_pool(name="ps", bufs=1, space="PSUM") as psp,
        ):
            ewt = sb.tile([P, BT], f32, name="ewt", tag="ewt")
            scr_v = sb.tile([P, 1], f32, name="scrv", tag="scrv")
            scr_s = sb.tile([1, BT], f32, name="scrs", tag="scrs")
            wxs = [
                sb.tile(
                    [P, (e - s) * CW + (EWPAD if ci == 0 else 0)],
                    bf16, name=f"wx{ci}", tag=f"wx{ci}",
                )
                for ci, (s, e) in enumerate(CHUNKS)
            ]
            y_v = sb.tile([P, BT * 512], bf16, name="yv", tag="yv")
            y_a = sb.tile([P, BT * 512], bf16, name="ya", tag="ya")
            pss = [
                [
                    psp.tile([P, 512], f32, name=f"ps{t}{h}", tag=f"ps{t}{h}")
                    for h in range(NH)
                ]
                for t in range(BT)
            ]

            # HAM warmers: matmuls over (uninitialized) y_v keep the PE
            # array busy from engine-boot until the first chunk lands, so
            # the clock-gate reaches 8/8 before the real matmuls start.
            # Their garbage output lands in bank (0,0), which the real
            # group's start=True clears.
            for _ in range(N_DUMMY):
                nc.tensor.matmul(
                    pss[0][0][0:1, :], y_v[:, 0:1], y_v[:, 0:512],
                    start=True, stop=True, skip_group_check=True,
                )

            # exactly 8 HWDGE DMAs in the whole kernel -> each DMAHW lane
            # is used once, so no DMA ever needs a lane-recycle wait on
            # top of its data wait (single-wait limit). wx0 first so the
            # PE's first real group is gated only by it; ew is not needed
            # until eviction.
            nc.sync.dma_start(wxs[0][:], wx0_d[:])
            for ci, (s, e) in enumerate(CHUNKS[1:], start=1):
                src = wxr_d[(s - 1) * P:(e - 1) * P, :].rearrange(
                    "(n p) c -> p n c", p=P
                )
                dst = wxs[ci][:].rearrange("p (n c) -> p n c", n=e - s)
                nc.sync.dma_start(dst, src)

            # accumulation: chunk-major so group n waits only on its
            # chunk's DMA; within a chunk, bank-major (t, h, n) so banks
            # finish staggered in the last chunk and evictions pipeline
            # behind the PE instead of serializing after it.
            for ci, (s, e) in enumerate(CHUNKS):
                wx = wxs[ci]
                for t in range(BT):
                    for n in range(s, e):
                        off = (n - s) * CW
                        lhsT = wx[
                            :, off + XOFF + P * t:off + XOFF + P * (t + 1)
                        ]
                        for h in range(NH):
                            nc.tensor.matmul(
                                pss[t][h][:], lhsT,
                                wx[:, off + 512 * h:off + 512 * (h + 1)],
                                start=(n == s and ci == 0),
                                stop=(n == e - 1 and ci == len(CHUNKS) - 1),
                                skip_group_check=(t == 0 and h == 0),
                            )

            # ew rides in chunk 0 as bf16; DVE upconverts it once (this
            # also absorbs the chunk-0 DMA wait for DVE), and the ACT
            # absorber reads the converted copy so real evictions carry
            # only their PE wait (single-wait limit)
            nc.vector.tensor_copy(ewt[:], wxs[0][:, CW:CW + BT])
            # absorber: reads ewt through the tensor_scalar ptr path so the
            # real DVE evicts don't carry a second (DVE-seq) wait
            nc.vector.tensor_scalar_mul(scr_v[:], wxs[0][:, 0:1], ewt[:, 0:1])
            nc.scalar.activation(scr_s[:], ewt[0:1, :], Copy)

            # evict: y[b,:] = ps[b,:] * ew[b]; DVE takes h=0, ACT h=1
            for t in range(BT):
                sc = ewt[:, t:t + 1]
                nc.vector.tensor_scalar_mul(
                    y_v[:, t * 512:(t + 1) * 512], pss[t][0][:], sc
                )
                nc.scalar.activation(
                    y_a[:, t * 512:(t + 1) * 512], pss[t][1][:], Copy, scale=sc
                )
            nc.sync.dma_start(yv_d[:, 0:1536], y_v[:, 0:1536])
            nc.sync.dma_start(yv_d[:, 1536:2048], y_v[:, 1536:2048])
            nc.sync.dma_start(ya_d[:], y_a[:])

    return nc


def _get_compiled():
    global _compiled
    if _compiled is None:
        _compiled = _build()
    return _compiled


_pack_cache = None


def _make_in_maps(x, expert_weights, weight, bias):
    global _pack_cache
    import ml_dtypes

    bf16 = ml_dtypes.bfloat16
    if _pack_cache is None or _pack_cache[0] is not weight:
        w = np.asarray(weight, dtype=np.float32)
        wx0s, wxrs = [], []
        for c in range(NCORES):
            wT = w[c].T.reshape(NIT, P, OUT).astype(bf16)  # [p,o]=W[c,o,128n+p]
            a0 = np.zeros((P, CW + EWPAD), dtype=bf16)
            a0[:, :OUT] = wT[0]
            ar = np.zeros((NIT - 1, P, CW), dtype=bf16)
            ar[:, :, :OUT] = wT[1:]
            wx0s.append(a0)
            wxrs.append(ar)
        _pack_cache = (weight, wx0s, wxrs)
    _, wx0s, wxrs = _pack_cache

    x = np.asarray(x, dtype=np.float32)
    ew = np.asarray(expert_weights, dtype=np.float32)
    # xT tile n: [p, b] = x[b, 128n+p]
    xTb = x.T.reshape(NIT, P, B).astype(bf16)
    in_maps = []
    for c in range(NCORES):
        wx0s[c][:, XOFF:XOFF + B] = xTb[0]
        wx0s[c][:, CW:CW + BT] = ew[:, c].reshape(BT, P).T.astype(bf16)
        wxrs[c][:, :, XOFF:] = xTb[1:]
        in_maps.append({
            "wx0": wx0s[c],
            "wxr": wxrs[c].reshape((NIT - 1) * P, CW),
        })
    return in_maps


def kernel(x, expert_weights, weight, bias, _trace=False):
    from concourse.bass_utils import run_bass_kernel_spmd

    nc = _get_compiled()
    in_maps = _make_in_maps(x, expert_weights, weight, bias)
    res = run_bass_kernel_spmd(
        nc, in_maps, core_ids=list(range(NCORES)), trace=_trace
    )
    acc = np.zeros((B, OUT), dtype=np.float32)
    for r in res.results:
        # yv[p, t*512+j] = y[128t+p, j]; ya[p, t*512+j] = y[128t+p, 512+j]
        yv = np.asarray(r["yv"], dtype=np.float32).reshape(P, BT, 512)
        ya = np.asarray(r["ya"], dtype=np.float32).reshape(P, BT, 512)
        acc[:, :512] += yv.transpose(1, 0, 2).reshape(B, 512)
        acc[:, 512:] += ya.transpose(1, 0, 2).reshape(B, 512)
    ew = np.asarray(expert_weights, dtype=np.float32)
    b = np.asarray(bias, dtype=np.float32)
    y = acc + ew @ b
    if _trace:
        return y, res
    return y
